# revision 14
# baseline (speedup 1.0000x reference)
"""Differential attention (DiffAttn) Trainium2 kernel, 8-core tensor-parallel.

Reference computation (per batch b, head h):
    q1,k1,q2,k2,v = x @ W*.T          (x: [B,S,D], W: [D,D], 16 heads x 128)
    a1 = softmax(q1 k1^T / sqrt(dh)); a2 = softmax(q2 k2^T / sqrt(dh))
    out = ((a1 - lam_h * a2) @ v) @ o_w.T

Sharding: tensor-parallel over heads. Core c owns heads {2c, 2c+1} (d_model
slice 256c:256c+256 of the projection outputs).  Each core computes a partial
o-projection output over its 256 input dims; the host sums the 8 partials.

Device-side layout choices:
  - x is passed pre-transposed (xt = x.T, [D, B*S]) so projections can run
    as  out.T[m, tok] = W_shard @ x.T  with the weight shard (host
    pre-transposed) as the stationary operand -> q/k tiles land in
    [head_dim(part), token(free)] layout, which feeds QK^T directly.
  - v is produced in natural [token, dim] layout (lhsT = x.T chunks) so it can
    be the stationary operand of the PV matmul.
  - scores are computed TRANSPOSED from the start: the k-block is the
    stationary operand and q streams, so the score tile lands as
    [key(part), query(free)].  exp'd tiles (E) then feed the PV matmul
    directly as the moving operand -- the PE-transpose pass of the previous
    design is gone entirely.
  - softmax sums in this layout run across partitions+tiles: a serial DVE
    accumulator T += E_kt trails the exp stream, then one ones-matmul
    reduces T across partitions into PSUM (every output row = the sum,
    broadcast for free).  r1 = 1/s1 and g = -lam*s1*r2 are [128, q] fp16 row
    vectors; E_comb = E1 + g*E2 in-place; the 1/s1 normalization rides the
    PV PSUM->SBUF copy as a tensor-tensor multiply.
  - attention is processed in q-halves of 1024 so the 32 E tiles fit SBUF.
  - all matmul inputs are bf16; PSUM accumulation is fp32; the o-proj output
    is stored bf16 (host accumulates the 8 partials in fp64).

Engine balance: attention is elementwise-bound (ACT exp, DVE accum/combine),
the projections are PE-bound.  The emission order software-pipelines them:

    proj(b0,h0)+v(b0) | attn(b0,h0) x proj(b0,h1) | attn(b0,h1) x proj(b1,h0)+v(b1)
    | attn(b1,h0) x proj(b1,h1) + oproj(b0)/2 | attn(b1,h1) x oproj(b0)/2; oproj(b1) drains
"""

import math

import numpy as np
import ml_dtypes

import concourse.bass as bass
import concourse.mybir as mybir
import concourse.tile as tile
from concourse import bass_utils

BF16 = mybir.dt.bfloat16
F16 = mybir.dt.float16
F32 = mybir.dt.float32

P = 128           # partitions / head_dim / PE tile
D = 2048          # d_model
B = 2
S = 2048          # seq len
T = B * S         # 4096 tokens
NH = 16           # total heads
NHL = 2           # heads per core
MD = NHL * P      # per-core projection dim (256)
KT = D // P       # 16 contraction tiles over d_model
ST = S // P       # 16 token tiles per batch
N_CORES = 8
CHUNK = 256       # token chunk for projection x streaming
QH = 1024         # q-half width for attention
QC = 512          # PV q-chunk
SCALE = 1.0 / math.sqrt(P)
LNSC = 2.0 ** -11          # pre-scale inside ln() so fp16 logs sit near 0
LNC = 11.0 * math.log(2.0)  # ln(1/LNSC)
QKN = ["wq1", "wk1", "wq2", "wk2"]
TACC_SPLIT = 8             # k-tiles 0..7 accumulate on DVE, 8..15 on GPSIMD
COMB_SPLIT = 4             # k-tiles 0..3 combine on GPSIMD, 4..15 on DVE

_mult = mybir.AluOpType.mult
_add = mybir.AluOpType.add


def _split_multi_waits(nc):
    """This walrus build accepts at most ONE sync-wait per instruction
    (codegen: "Too many sync wait commands").  Tile attaches one wait per
    upstream proc, so split the extras onto same-engine NOP carriers placed
    immediately before the instruction — the engine stalls on each carrier in
    turn, which is sequentially equivalent."""
    n = 0
    for bb in nc.main_func.blocks:
        out = []
        for ins in bb.instructions:
            si = getattr(ins, "sync_info", None)
            waits = list(si.on_wait) if si is not None and si.on_wait else []
            if len(waits) > 1:
                for w in waits[:-1]:
                    n += 1
                    out.append(
                        mybir.InstNoOp(
                            name=f"{ins.name}-wsplit{n}",
                            engine=ins.engine,
                            sync_info=mybir.SyncInfo(on_wait=[w], on_update=[]),
                            bass_nofuse=True,
                        )
                    )
                si.on_wait = waits[-1:]
            out.append(ins)
        bb.instructions[:] = out


class Kern:
    """Holds pools/constants; methods emit one group of instructions each.
    The driver (build) calls them in a software-pipelined order."""

    def __init__(self, nc, tc, pools):
        self.nc = nc
        self.tc = tc
        (self.cpool, self.projpool, self.xpool, self.apool,
         self.ps_score, self.ps_mm, self.ps_sum) = pools
        self.qk = {}      # (b, n, h) -> tile (slots shared across b via tags)
        self.vbuf = {}    # b -> tile
        self.aoT = {}     # (b, h) -> tile
        self.xc_cur = None
        self.w_sb = {}

    def load_w(self, w_d, n, split=False):
        t = self.cpool.tile([P, KT, MD], BF16, name=f"{n}_sb")
        src_ap = w_d[n].rearrange("p (kt m) -> p kt m", m=MD)
        if split:
            self.nc.sync.dma_start(t[:, : KT // 2], src_ap[:, : KT // 2])
            self.nc.sync.dma_start(t[:, KT // 2 :], src_ap[:, KT // 2 :])
        else:
            self.nc.sync.dma_start(t, src_ap)
        self.w_sb[n] = t

    def load_consts(self, loglam_d, ones_d):
        nc = self.nc
        self.loglam_sb = self.cpool.tile([P, NHL + 1], F32, name="loglam_sb")
        nc.sync.dma_start(self.loglam_sb, loglam_d.ap())
        self.ones_sb = self.cpool.tile([P, P], BF16, name="ones_sb")
        nc.sync.dma_start(self.ones_sb, ones_d.ap())

    def load_wo(self, wo_d):
        self.wo_sb = self.cpool.tile([P, NHL, D], BF16, name="wo_sb")
        self.nc.sync.dma_start(self.wo_sb, wo_d.rearrange("p (h n) -> p h n", n=D))

    # ---- projection pieces ----
    def load_xc(self, xt, b, ci, half=None):
        tok0 = b * S + ci * CHUNK
        if half is None:
            xc = self.xpool.tile([P, KT, CHUNK], BF16, name="xc", tag="xc")
            self.nc.sync.dma_start(
                xc, xt[:, tok0 : tok0 + CHUNK].rearrange("(kt p) n -> p kt n", p=P)
            )
            self.xc_cur = xc
        else:
            # first-chunk fast path: land each half as its own DMA so the
            # first matmuls only wait on 0.5MB of x
            if half == 0:
                self.xc_cur = self.xpool.tile([P, KT, CHUNK], BF16, name="xc", tag="xc")
            hw = CHUNK // 2
            self.nc.sync.dma_start(
                self.xc_cur[:, :, half * hw : (half + 1) * hw],
                xt[:, tok0 + half * hw : tok0 + (half + 1) * hw].rearrange(
                    "(kt p) n -> p kt n", p=P
                ),
            )

    def proj_qk_group(self, b, n, h, ci, half=None):
        """One (weight, head) projection for one CHUNK-token chunk: 16 matmuls
        accumulated in PSUM, then a DVE cast into the persistent qk tile."""
        nc = self.nc
        key = (b, n, h)
        if key not in self.qk:
            self.qk[key] = self.projpool.tile(
                [P, S], BF16, name=f"{n}h{h}b{b}", tag=f"{n}h{h}"
            )
        ps = self.ps_mm.tile([P, 512], F32, name="psp", tag="mm")
        xc = self.xc_cur
        lo, sz = (0, CHUNK) if half is None else (half * (CHUNK // 2), CHUNK // 2)
        for kt in range(KT):
            nc.tensor.matmul(
                ps[:, :sz],
                lhsT=self.w_sb[n][:, kt, h * P : (h + 1) * P],
                rhs=xc[:, kt, lo : lo + sz],
                start=(kt == 0),
                stop=(kt == KT - 1),
            )
        # alternate the PSUM->SBUF cast between DVE and ACT to balance load
        dst = self.qk[key][:, ci * CHUNK + lo : ci * CHUNK + lo + sz]
        if ci % 2 == 0:
            nc.vector.tensor_copy(dst, ps[:, :sz])
        else:
            nc.scalar.copy(dst, ps[:, :sz])

    def proj_v_group(self, b, ci, s4):
        nc = self.nc
        if b not in self.vbuf:
            self.vbuf[b] = self.projpool.tile(
                [P, ST, MD], BF16, name=f"vbuf{b}", tag=f"vbuf{b}"
            )
        ps = self.ps_mm.tile([P, 512], F32, name="psv", tag="mm")
        xc = self.xc_cur
        for kt in range(KT):
            nc.tensor.matmul(
                ps[:, :MD],
                lhsT=xc[:, kt, s4 * P : (s4 + 1) * P],
                rhs=self.w_sb["wv"][:, kt, :],
                start=(kt == 0),
                stop=(kt == KT - 1),
            )
        dst = self.vbuf[b][:, ci * (CHUNK // P) + s4, :]
        if s4 % 2 == 0:
            nc.vector.tensor_copy(dst, ps[:, :MD])
        else:
            nc.scalar.copy(dst, ps[:, :MD])

    # ---- attention pieces (scores in [k, q] layout) ----
    def attn_qk_kt(self, b, si, h, half, kt, ebuf, tacc_d, tacc_g):
        """Scores+exp for one (softmax, k-tile): psum[k=128, q=1024] via two
        512-wide matmuls with the k-block stationary, exp into ebuf[:, kt, :],
        and serial sum-accumulators T += E_kt (k-tiles < TACC_SPLIT on DVE,
        the rest on the otherwise-idle GPSIMD)."""
        nc = self.nc
        qn, kn = ("wq1", "wk1") if si == 0 else ("wq2", "wk2")
        qh_t = self.qk[(b, qn, h)]
        kh_t = self.qk[(b, kn, h)]
        qlo = half * QH
        ps = self.ps_score.tile([P, QH], F32, name="pss", tag="score")
        for j in range(QH // 512):
            nc.tensor.matmul(
                ps[:, j * 512 : (j + 1) * 512],
                lhsT=kh_t[:, kt * P : (kt + 1) * P],
                rhs=qh_t[:, qlo + j * 512 : qlo + (j + 1) * 512],
                start=True,
                stop=True,
            )
        nc.scalar.activation(
            ebuf[:, kt, :], ps, mybir.ActivationFunctionType.Exp, scale=SCALE
        )
        if kt == 1:
            nc.vector.tensor_add(tacc_d, ebuf[:, 0, :], ebuf[:, 1, :])
        elif 1 < kt < TACC_SPLIT:
            nc.vector.tensor_add(tacc_d, tacc_d, ebuf[:, kt, :])
        elif kt == TACC_SPLIT + 1:
            nc.gpsimd.tensor_add(tacc_g, ebuf[:, TACC_SPLIT, :], ebuf[:, kt, :])
        elif kt > TACC_SPLIT + 1:
            nc.gpsimd.tensor_add(tacc_g, tacc_g, ebuf[:, kt, :])

    def attn_sums(self, si, h, tacc_d, tacc_g, ltile):
        """Partition-reduce the two T accumulators via ones-matmuls into one
        PSUM (broadcast sum), then L = ln(s * 2^-11) in fp16 on ACT.  For
        softmax-1 additionally d = L1 - L2, g = exp(d + ln lam) (positive;
        the combine subtracts), and r1 = exp(-L1 - ln 2^11) = 1/s1."""
        nc = self.nc
        pssum = self.ps_sum.tile([P, QH], F32, name="pssum", tag="sum")
        for j in range(QH // 512):
            sl = slice(j * 512, (j + 1) * 512)
            nc.tensor.matmul(pssum[:, sl], lhsT=self.ones_sb, rhs=tacc_d[:, sl],
                             start=True, stop=False)
            nc.tensor.matmul(pssum[:, sl], lhsT=self.ones_sb, rhs=tacc_g[:, sl],
                             start=False, stop=True)
        nc.scalar.activation(ltile, pssum, mybir.ActivationFunctionType.Ln,
                             scale=LNSC)
        if si == 0:
            with nc.allow_low_precision(reason="fp16 softmax normalizers"):
                d = self.apool.tile([P, QH], F16, name="dln", tag="dln", bufs=1)
                nc.vector.tensor_sub(d, ltile, self.l2_cur)
                g = self.apool.tile([P, QH], F16, name="g", tag="g", bufs=1)
                nc.scalar.activation(g, d, mybir.ActivationFunctionType.Exp,
                                     bias=self.loglam_sb[:, h : h + 1])
                self.g_cur = g
                r1 = self.apool.tile([P, QH], F16, name="r1", tag="r1", bufs=1)
                nc.scalar.activation(r1, ltile,
                                     mybir.ActivationFunctionType.Exp,
                                     scale=-1.0,
                                     bias=self.loglam_sb[:, NHL : NHL + 1])
                self.r1_cur = r1

    def attn_combine_kt(self, e1buf, e2buf, kt):
        """E2 *= g; E1 -= E2 (in place, 1024 wide; low k-tiles on GPSIMD)."""
        nc = self.nc
        eng = nc.gpsimd if kt < COMB_SPLIT else nc.vector
        eng.tensor_mul(e2buf[:, kt, :], e2buf[:, kt, :], self.g_cur)
        eng.tensor_sub(e1buf[:, kt, :], e1buf[:, kt, :], e2buf[:, kt, :])

    def attn_pv(self, b, h, half, qc, e1buf):
        nc = self.nc
        if (b, h) not in self.aoT:
            self.aoT[(b, h)] = self.projpool.tile(
                [P, S], BF16, name=f"aoT{b}{h}", tag=f"aoT{b}{h}"
            )
        pso = self.ps_mm.tile([P, 512], F32, name="pso", tag="mm")
        for kt in range(ST):
            nc.tensor.matmul(
                pso[:, :QC],
                lhsT=self.vbuf[b][:, kt, h * P : (h + 1) * P],
                rhs=e1buf[:, kt, qc * QC : (qc + 1) * QC],
                start=(kt == 0),
                stop=(kt == ST - 1),
            )
        # normalization by 1/s1 rides the PSUM->SBUF copy
        nc.vector.tensor_mul(
            self.aoT[(b, h)][:, half * QH + qc * QC : half * QH + (qc + 1) * QC],
            pso[:, :QC],
            self.r1_cur[:, qc * QC : (qc + 1) * QC],
        )

    # ---- o-projection piece ----
    def oproj_tt(self, out_d, b, tt):
        nc = self.nc
        ob = self.xpool.tile([P, D], BF16, name="ob", tag="xc")
        for nq in range(D // 512):
            ps = self.ps_mm.tile([P, 512], F32, name="pso2", tag="mm")
            for h in range(NHL):
                nc.tensor.matmul(
                    ps,
                    lhsT=self.aoT[(b, h)][:, tt * P : (tt + 1) * P],
                    rhs=self.wo_sb[:, h, nq * 512 : (nq + 1) * 512],
                    start=(h == 0),
                    stop=(h == NHL - 1),
                )
            if nq % 2 == 0:
                nc.vector.tensor_copy(ob[:, nq * 512 : (nq + 1) * 512], ps)
            else:
                nc.scalar.copy(ob[:, nq * 512 : (nq + 1) * 512], ps)
            if nq % 2 == 1:
                nc.sync.dma_start(
                    out_d[
                        b * S + tt * P : b * S + (tt + 1) * P,
                        (nq - 1) * 512 : (nq + 1) * 512,
                    ],
                    ob[:, (nq - 1) * 512 : (nq + 1) * 512],
                )


def _proj_slot_groups(k, xt, b, h, with_v, skip_first_xc=False,
                      skip_first_wq1=False):
    """Yield emission closures for one head-slot's projections (chunks x
    4 weights, plus optionally the v projection groups)."""
    for ci in range(S // CHUNK):
        if not (skip_first_xc and ci == 0):
            yield lambda ci=ci: k.load_xc(xt, b, ci)
        for n in QKN:
            if skip_first_wq1 and ci == 0 and n == "wq1":
                continue
            yield lambda n=n, ci=ci: k.proj_qk_group(b, n, h, ci)
        if with_v:
            for s4 in range(CHUNK // P):
                yield lambda ci=ci, s4=s4: k.proj_v_group(b, ci, s4)


def _attn_head(k, b, h, bg_iter, post_pv=None):
    """Emit one head's attention (two q-halves), interleaving background
    closures (projections of the next head-slot / o-projection token tiles)
    between emission steps.  post_pv(qc_global) yields closures that depend on
    this head's PV output for 512-token chunk qc_global (the final
    o-projection); they are paced after subsequent steps."""
    nc = k.nc
    n_bg = getattr(bg_iter, "length", 0)
    emitted = 0
    pending = []
    steps_total = 2 * (2 * (KT + 1) + KT + 2 + 2)   # per-half emission steps
    steps_total *= 1
    step = 0
    total_steps = 2 * (2 * (KT + 1) + KT + 4)

    def bg_tick():
        nonlocal emitted, step
        step += 1
        if pending:
            pending.pop(0)()
        want = (step * n_bg) // total_steps
        while emitted < want:
            next(bg_iter.it)()
            emitted += 1

    for half in range(2):
        e1 = k.apool.tile([P, KT, QH], BF16, name="e1", tag="e1", bufs=1)
        e2 = k.apool.tile([P, KT, QH], BF16, name="e2", tag="e2", bufs=1)
        l2 = k.apool.tile([P, QH], F16, name="l2", tag="l2", bufs=1)
        l1 = k.apool.tile([P, QH], F16, name="l1", tag="l1", bufs=1)
        k.l2_cur = l2
        # softmax-2 first (its sum feeds g), then softmax-1
        for si, ebuf, lt in ((1, e2, l2), (0, e1, l1)):
            tacc_d = k.apool.tile([P, QH], BF16, name="taccd", tag="taccd", bufs=1)
            tacc_g = k.apool.tile([P, QH], BF16, name="taccg", tag="taccg", bufs=1)
            for kt in range(KT):
                k.attn_qk_kt(b, si, h, half, kt, ebuf, tacc_d, tacc_g)
                bg_tick()
            k.attn_sums(si, h, tacc_d, tacc_g, lt)
            bg_tick()
        for kt in range(KT):
            k.attn_combine_kt(e1, e2, kt)
            bg_tick()
        for qc in range(QH // QC):
            k.attn_pv(b, h, half, qc, e1)
            bg_tick()
            qc_global = half * (QH // QC) + qc
            if post_pv is not None:
                pending.extend(post_pv(qc_global))
            bg_tick()
    for fn in pending:
        fn()
    while emitted < n_bg:
        next(bg_iter.it)()
        emitted += 1


class _BG:
    def __init__(self, gens):
        items = [g for gen in gens for g in gen]
        self.it = iter(items)
        self.length = len(items)


def build_nc():
    nc = bass.Bass("TRN2", target_bir_lowering=False, debug=False)

    xt = nc.dram_tensor("xt", [D, T], BF16, kind="ExternalInput")
    wnames = ["wq1", "wk1", "wq2", "wk2", "wv"]
    w_d = {n: nc.dram_tensor(n, [P, KT * MD], BF16, kind="ExternalInput") for n in wnames}
    wo_d = nc.dram_tensor("wo", [P, NHL * D], BF16, kind="ExternalInput")
    loglam_d = nc.dram_tensor("loglam", [P, NHL + 1], F32, kind="ExternalInput")
    ones_d = nc.dram_tensor("ones", [P, P], BF16, kind="ExternalInput")
    out_d = nc.dram_tensor("out", [T, D], BF16, kind="ExternalOutput")

    with tile.TileContext(nc) as tc:
        with (
            tc.tile_pool(name="const", bufs=1) as cpool,
            tc.tile_pool(name="proj", bufs=1) as projpool,
            tc.tile_pool(name="xchunk", bufs=2) as xpool,
            tc.tile_pool(name="attn", bufs=2) as apool,
            tc.tile_pool(name="ps_score", bufs=2, space="PSUM") as ps_score,
            tc.tile_pool(name="ps_mm", bufs=2, space="PSUM") as ps_mm,
            tc.tile_pool(name="ps_sum", bufs=1, space="PSUM") as ps_sum,
        ):
            k = Kern(nc, tc, (cpool, projpool, xpool, apool,
                              ps_score, ps_mm, ps_sum))
            # DMA queues drain in emission order: put the first half-chunk of
            # x and the first-used weight at the head of the line so the PE
            # starts as soon as ~1.5MB have landed.
            k.load_xc(xt, 0, 0, half=0)
            k.load_w(w_d, "wq1", split=True)
            k.load_xc(xt, 0, 0, half=1)
            for n in ["wk1", "wq2", "wk2", "wv"]:
                k.load_w(w_d, n)
            k.load_consts(loglam_d, ones_d)

            # prologue: batch-0 head-0 projections + v(b0); the first chunk's
            # q1 projection runs as two half-width groups
            k.proj_qk_group(0, "wq1", 0, 0, half=0)
            k.proj_qk_group(0, "wq1", 0, 0, half=1)
            for fn in _proj_slot_groups(k, xt, 0, 0, with_v=True, skip_first_xc=True,
                                        skip_first_wq1=True):
                fn()
            k.load_wo(wo_d)
            # attn(b0,h0) x proj(b0,h1)
            _attn_head(k, 0, 0, _BG([_proj_slot_groups(k, xt, 0, 1, False)]))
            # attn(b0,h1) x proj(b1,h0)+v(b1)
            _attn_head(k, 0, 1, _BG([_proj_slot_groups(k, xt, 1, 0, True)]))
            # attn(b1,h0) x proj(b1,h1) + first half of oproj(b0)
            _attn_head(k, 1, 0, _BG([
                _proj_slot_groups(k, xt, 1, 1, False),
                [(lambda tt=tt: k.oproj_tt(out_d, 0, tt)) for tt in range(ST // 2)],
            ]))
            # attn(b1,h1) x oproj(b0); oproj(b1,tt) drains right after the PV
            # that completes its aoT columns, leaving almost no tail.
            def _drain_oproj_b1(qc):
                return [
                    (lambda tt=tt: k.oproj_tt(out_d, 1, tt))
                    for tt in range(4 * qc, 4 * qc + 4)
                ]

            _attn_head(
                k, 1, 1,
                _BG([[(lambda tt=tt: k.oproj_tt(out_d, 0, tt)) for tt in range(ST // 2, ST)]]),
                post_pv=_drain_oproj_b1,
            )

    _split_multi_waits(nc)
    return nc


_NC_CACHE = None


def _get_nc():
    global _NC_CACHE
    if _NC_CACHE is None:
        _NC_CACHE = build_nc()
    return _NC_CACHE


def _wlay(w_shard):
    """[MD, D] weight shard -> W.T laid out as the SBUF tile [128, KT*MD]."""
    bf = ml_dtypes.bfloat16
    wt = w_shard.T                                   # [D, MD]
    return np.ascontiguousarray(
        wt.reshape(KT, P, MD).transpose(1, 0, 2).reshape(P, KT * MD)
    ).astype(bf)


def _wolay(wo_shard):
    """[D, MD] o_w columns -> O.T laid out as the SBUF tile [128, NHL*D]."""
    bf = ml_dtypes.bfloat16
    wt = wo_shard.T                                  # [MD, D]
    return np.ascontiguousarray(
        wt.reshape(NHL, P, D).transpose(1, 0, 2).reshape(P, NHL * D)
    ).astype(bf)


def make_in_maps(inputs):
    bf = ml_dtypes.bfloat16
    x = np.asarray(inputs["x"], np.float32)
    lam = np.asarray(inputs["lambda_param"], np.float32)
    xt = np.ascontiguousarray(x.reshape(T, D).T).astype(bf)
    ones = np.ones((P, P), dtype=bf)

    in_maps = []
    for c in range(N_CORES):
        hs = slice(c * MD, (c + 1) * MD)
        m = {
            "xt": xt,
            "wq1": _wlay(np.asarray(inputs["q1_w"], np.float32)[hs, :]),
            "wk1": _wlay(np.asarray(inputs["k1_w"], np.float32)[hs, :]),
            "wq2": _wlay(np.asarray(inputs["q2_w"], np.float32)[hs, :]),
            "wk2": _wlay(np.asarray(inputs["k2_w"], np.float32)[hs, :]),
            "wv": _wlay(np.asarray(inputs["v_w"], np.float32)[hs, :]),
            "wo": _wolay(np.asarray(inputs["o_w"], np.float32)[:, hs]),
            "loglam": np.tile(np.concatenate([np.log(lam[c * NHL : (c + 1) * NHL]), [-LNC]])[None, :], (P, 1)).astype(np.float32),
            "ones": ones,
        }
        in_maps.append(m)
    return in_maps


def kernel(**inputs):
    in_maps = make_in_maps(inputs)
    nc = _get_nc()
    res = bass_utils.run_bass_kernel_spmd(nc, in_maps, core_ids=list(range(N_CORES)))
    acc = np.zeros((T, D), np.float64)
    for r in res.results:
        acc += np.asarray(r["out"], np.float64)
    return acc.reshape(B, S, D).astype(np.float32)


if __name__ == "__main__":
    nc = build_nc()
    print("built OK")


# revision 17
# speedup vs baseline: 1.4739x; 1.4739x over previous
"""Differential attention (DiffAttn) Trainium2 kernel, 8-core tensor-parallel.

Reference computation (per batch b, head h):
    q1,k1,q2,k2,v = x @ W*.T          (x: [B,S,D], W: [D,D], 16 heads x 128)
    a1 = softmax(q1 k1^T / sqrt(dh)); a2 = softmax(q2 k2^T / sqrt(dh))
    out = ((a1 - lam_h * a2) @ v) @ o_w.T

Sharding: tensor-parallel over heads. Core c owns heads {2c, 2c+1} (d_model
slice 256c:256c+256 of the projection outputs).  Each core computes a partial
o-projection output over its 256 input dims; the host sums the 8 partials.

Device-side layout choices:
  - x is passed pre-transposed (xt = x.T, [D, B*S]) so projections can run
    as  out.T[m, tok] = W_shard @ x.T  with the weight shard (host
    pre-transposed) as the stationary operand -> q/k tiles land in
    [head_dim(part), token(free)] layout, which feeds QK^T directly.
  - v is produced in natural [token, dim] layout (lhsT = x.T chunks) so it can
    be the stationary operand of the PV matmul.
  - scores are computed TRANSPOSED from the start: the k-block is the
    stationary operand and q streams, so the score tile lands as
    [key(part), query(free)].  exp'd tiles (E) then feed the PV matmul
    directly as the moving operand -- the PE-transpose pass of the previous
    design is gone entirely.
  - softmax sums in this layout run across partitions+tiles: a serial DVE
    accumulator T += E_kt trails the exp stream, then one ones-matmul
    reduces T across partitions into PSUM (every output row = the sum,
    broadcast for free).  r1 = 1/s1 and g = -lam*s1*r2 are [128, q] fp16 row
    vectors; E_comb = E1 + g*E2 in-place; the 1/s1 normalization rides the
    PV PSUM->SBUF copy as a tensor-tensor multiply.
  - attention is processed in q-halves of 1024 so the 32 E tiles fit SBUF.
  - all matmul inputs are bf16; PSUM accumulation is fp32; the o-proj output
    is stored bf16 (host accumulates the 8 partials in fp64).

Engine balance: attention is elementwise-bound (ACT exp, DVE accum/combine),
the projections are PE-bound.  The emission order software-pipelines them:

    proj(b0,h0)+v(b0) | attn(b0,h0) x proj(b0,h1) | attn(b0,h1) x proj(b1,h0)+v(b1)
    | attn(b1,h0) x proj(b1,h1) + oproj(b0)/2 | attn(b1,h1) x oproj(b0)/2; oproj(b1) drains
"""

import math

import numpy as np
import ml_dtypes

import concourse.bass as bass
import concourse.mybir as mybir
import concourse.tile as tile
from concourse import bass_utils

BF16 = mybir.dt.bfloat16
F16 = mybir.dt.float16
F8 = mybir.dt.float8e4
F32 = mybir.dt.float32

P = 128           # partitions / head_dim / PE tile
D = 2048          # d_model
B = 2
S = 2048          # seq len
T = B * S         # 4096 tokens
NH = 16           # total heads
NHL = 2           # heads per core
MD = NHL * P      # per-core projection dim (256)
KT = D // P       # 16 contraction tiles over d_model
ST = S // P       # 16 token tiles per batch
N_CORES = 8
CHUNK = 256       # token chunk for projection x streaming
QH = 1024         # q-half width for attention
QC = 512          # PV q-chunk
SCALE = 1.0 / math.sqrt(P)
LNSC = 2.0 ** -11          # pre-scale inside ln() so fp16 logs sit near 0
LNC = 11.0 * math.log(2.0)  # ln(1/LNSC)
QKN = ["wq1", "wk1", "wq2", "wk2"]
TACC_SPLIT = 8             # k-tiles 0..7 accumulate on DVE, 8..15 on GPSIMD
COMB_SPLIT = 4             # k-tiles 0..3 combine on GPSIMD, 4..15 on DVE

_mult = mybir.AluOpType.mult
_add = mybir.AluOpType.add


def _split_multi_waits(nc):
    """This walrus build accepts at most ONE sync-wait per instruction
    (codegen: "Too many sync wait commands").  Tile attaches one wait per
    upstream proc, so split the extras onto same-engine NOP carriers placed
    immediately before the instruction — the engine stalls on each carrier in
    turn, which is sequentially equivalent."""
    n = 0
    for bb in nc.main_func.blocks:
        out = []
        for ins in bb.instructions:
            si = getattr(ins, "sync_info", None)
            waits = list(si.on_wait) if si is not None and si.on_wait else []
            if len(waits) > 1:
                for w in waits[:-1]:
                    n += 1
                    out.append(
                        mybir.InstNoOp(
                            name=f"{ins.name}-wsplit{n}",
                            engine=ins.engine,
                            sync_info=mybir.SyncInfo(on_wait=[w], on_update=[]),
                            bass_nofuse=True,
                        )
                    )
                si.on_wait = waits[-1:]
            out.append(ins)
        bb.instructions[:] = out


class Kern:
    """Holds pools/constants; methods emit one group of instructions each.
    The driver (build) calls them in a software-pipelined order."""

    def __init__(self, nc, tc, pools):
        self.nc = nc
        self.tc = tc
        (self.cpool, self.projpool, self.xpool, self.apool,
         self.ps_score, self.ps_mm, self.ps_sum) = pools
        self.qk = {}      # (b, n, h) -> tile (slots shared across b via tags)
        self.vbuf = {}    # b -> tile
        self.vbuf8 = {}   # b -> fp8 copy for the DoubleRow PV2
        self.aoT = {}     # (b, h) -> tile
        self.xc_cur = None
        self.w_sb = {}

    def load_w(self, w_d, n, split=False):
        t = self.cpool.tile([P, KT, MD], BF16, name=f"{n}_sb")
        src_ap = w_d[n].rearrange("p (kt m) -> p kt m", m=MD)
        if split:
            self.nc.sync.dma_start(t[:, : KT // 2], src_ap[:, : KT // 2])
            self.nc.sync.dma_start(t[:, KT // 2 :], src_ap[:, KT // 2 :])
        else:
            self.nc.sync.dma_start(t, src_ap)
        self.w_sb[n] = t

    def load_consts(self, loglam_d, ones_d):
        nc = self.nc
        self.loglam_sb = self.cpool.tile([P, NHL + 1], F32, name="loglam_sb")
        nc.sync.dma_start(self.loglam_sb, loglam_d.ap())
        self.ones_sb = self.cpool.tile([P, P], BF16, name="ones_sb")
        nc.sync.dma_start(self.ones_sb, ones_d.ap())

    def load_wo(self, wo_d):
        self.wo_sb = self.cpool.tile([P, NHL, D], BF16, name="wo_sb")
        self.nc.sync.dma_start(self.wo_sb, wo_d.rearrange("p (h n) -> p h n", n=D))

    # ---- projection pieces ----
    def load_xc(self, xt, b, ci, half=None):
        tok0 = b * S + ci * CHUNK
        if half is None:
            xc = self.xpool.tile([P, KT, CHUNK], BF16, name="xc", tag="xc")
            self.nc.sync.dma_start(
                xc, xt[:, tok0 : tok0 + CHUNK].rearrange("(kt p) n -> p kt n", p=P)
            )
            self.xc_cur = xc
        else:
            # first-chunk fast path: land each half as its own DMA so the
            # first matmuls only wait on 0.5MB of x
            if half == 0:
                self.xc_cur = self.xpool.tile([P, KT, CHUNK], BF16, name="xc", tag="xc")
            hw = CHUNK // 2
            self.nc.sync.dma_start(
                self.xc_cur[:, :, half * hw : (half + 1) * hw],
                xt[:, tok0 + half * hw : tok0 + (half + 1) * hw].rearrange(
                    "(kt p) n -> p kt n", p=P
                ),
            )

    def proj_qk_group(self, b, n, h, ci, half=None):
        """One (weight, head) projection for one CHUNK-token chunk: 16 matmuls
        accumulated in PSUM, then a DVE cast into the persistent qk tile."""
        nc = self.nc
        key = (b, n, h)
        if key not in self.qk:
            self.qk[key] = self.projpool.tile(
                [P, S], BF16, name=f"{n}h{h}b{b}", tag=f"{n}h{h}"
            )
        ps = self.ps_mm.tile([P, 512], F32, name="psp", tag="mm")
        xc = self.xc_cur
        lo, sz = (0, CHUNK) if half is None else (half * (CHUNK // 2), CHUNK // 2)
        for kt in range(KT):
            nc.tensor.matmul(
                ps[:, :sz],
                lhsT=self.w_sb[n][:, kt, h * P : (h + 1) * P],
                rhs=xc[:, kt, lo : lo + sz],
                start=(kt == 0),
                stop=(kt == KT - 1),
            )
        nc.vector.tensor_copy(
            self.qk[key][:, ci * CHUNK + lo : ci * CHUNK + lo + sz], ps[:, :sz]
        )

    def proj_v_group(self, b, ci, s4):
        nc = self.nc
        if b not in self.vbuf:
            self.vbuf[b] = self.projpool.tile(
                [P, ST, MD], BF16, name=f"vbuf{b}", tag=f"vbuf{b}"
            )
            self.vbuf8[b] = self.projpool.tile(
                [P, ST, MD], F8, name=f"vbuf8{b}", tag=f"vbuf8{b}"
            )
        ps = self.ps_mm.tile([P, 512], F32, name="psv", tag="mm")
        xc = self.xc_cur
        for kt in range(KT):
            nc.tensor.matmul(
                ps[:, :MD],
                lhsT=xc[:, kt, s4 * P : (s4 + 1) * P],
                rhs=self.w_sb["wv"][:, kt, :],
                start=(kt == 0),
                stop=(kt == KT - 1),
            )
        blk = ci * (CHUNK // P) + s4
        nc.vector.tensor_copy(self.vbuf[b][:, blk, :], ps[:, :MD])
        nc.scalar.copy(self.vbuf8[b][:, blk, :], ps[:, :MD])

    # ---- attention pieces (scores in [k, q] layout) ----
    def attn_qk_kt(self, b, si, h, half, kt, ebuf, tacc):
        """Scores+exp for one (softmax, k-tile): psum[k=128, q=1024] via two
        512-wide matmuls with the k-block stationary, exp into ebuf[:, kt, :]
        (bf16 for softmax-1, fp8 for softmax-2), and the serial DVE
        sum-accumulator T += E_kt trailing the exp stream."""
        nc = self.nc
        qn, kn = ("wq1", "wk1") if si == 0 else ("wq2", "wk2")
        qh_t = self.qk[(b, qn, h)]
        kh_t = self.qk[(b, kn, h)]
        qlo = half * QH
        ps = self.ps_score.tile([P, QH], F32, name="pss", tag="score")
        for j in range(QH // 512):
            nc.tensor.matmul(
                ps[:, j * 512 : (j + 1) * 512],
                lhsT=kh_t[:, kt * P : (kt + 1) * P],
                rhs=qh_t[:, qlo + j * 512 : qlo + (j + 1) * 512],
                start=True,
                stop=True,
            )
        nc.scalar.activation(
            ebuf[:, kt, :], ps, mybir.ActivationFunctionType.Exp, scale=SCALE
        )
        if kt == 1:
            nc.vector.tensor_add(tacc, ebuf[:, 0, :], ebuf[:, 1, :])
        elif kt > 1:
            nc.vector.tensor_add(tacc, tacc, ebuf[:, kt, :])

    def attn_sums(self, si, h, tacc, ltile):
        """Partition-reduce T via a ones-matmul into PSUM (broadcast sum),
        then L = ln(s * 2^-11) fp16 on ACT, and the fp16 column scales:
        softmax-2: gl2 = exp(-L2 + ln(lam) - LNC) = lam/s2
        softmax-1: r1  = exp(-L1 - LNC) = 1/s1"""
        nc = self.nc
        pssum = self.ps_score.tile([P, QH], F32, name="pssum", tag="score")
        for j in range(QH // 512):
            sl = slice(j * 512, (j + 1) * 512)
            nc.tensor.matmul(pssum[:, sl], lhsT=self.ones_sb, rhs=tacc[:, sl],
                             start=True, stop=True)
        nc.scalar.activation(ltile, pssum, mybir.ActivationFunctionType.Ln,
                             scale=LNSC)
        with nc.allow_low_precision(reason="fp16 softmax normalizers"):
            if si == 1:
                gl2 = self.apool.tile([P, QH], F16, name="gl2", tag="gl2", bufs=1)
                nc.scalar.activation(gl2, ltile,
                                     mybir.ActivationFunctionType.Exp,
                                     scale=-1.0,
                                     bias=self.loglam_sb[:, h : h + 1])
                self.gl2_cur = gl2
            else:
                r1 = self.apool.tile([P, QH], F16, name="r1", tag="r1", bufs=1)
                nc.scalar.activation(r1, ltile,
                                     mybir.ActivationFunctionType.Exp,
                                     scale=-1.0,
                                     bias=self.loglam_sb[:, NHL : NHL + 1])
                self.r1_cur = r1

    def attn_pv2(self, b, h, qc, e2buf):
        """lam-weighted softmax-2 PV in fp8 DoubleRow: contracts k-tile PAIRS
        (256 per matmul) at 2x rate.  Result is held in PSUM (tag pv2) until
        the merge after PV1."""
        nc = self.nc
        pso2 = self.ps_mm.tile([P, 512], F32, name="pso2", tag="pv2", bufs=2)
        for t in range(ST // 2):
            nc.tensor.matmul(
                pso2[:, :QC],
                lhsT=self.vbuf8[b][:, 2 * t : 2 * t + 2, h * P : (h + 1) * P],
                rhs=e2buf[:, 2 * t : 2 * t + 2, qc * QC : (qc + 1) * QC],
                start=(t == 0),
                stop=(t == ST // 2 - 1),
                perf_mode=mybir.MatmulPerfMode.DoubleRow,
            )
        return pso2

    def attn_pv1_merge(self, b, h, half, qc, e1buf, pso2):
        """Softmax-1 PV (bf16) + merge: aoT = pso1*r1 - pso2*gl2."""
        nc = self.nc
        if (b, h) not in self.aoT:
            self.aoT[(b, h)] = self.projpool.tile(
                [P, S], BF16, name=f"aoT{b}{h}", tag=f"aoT{b}{h}"
            )
        pso1 = self.ps_mm.tile([P, 512], F32, name="pso1", tag="mm")
        for kt in range(ST):
            nc.tensor.matmul(
                pso1[:, :QC],
                lhsT=self.vbuf[b][:, kt, h * P : (h + 1) * P],
                rhs=e1buf[:, kt, qc * QC : (qc + 1) * QC],
                start=(kt == 0),
                stop=(kt == ST - 1),
            )
        dst = self.aoT[(b, h)][:, half * QH + qc * QC : half * QH + (qc + 1) * QC]
        ta = self.apool.tile([P, QC], BF16, name="ta", tag="ta", bufs=2)
        nc.vector.tensor_mul(ta, pso2[:, :QC], self.gl2_cur[:, qc * QC : (qc + 1) * QC])
        nc.vector.tensor_mul(dst, pso1[:, :QC], self.r1_cur[:, qc * QC : (qc + 1) * QC])
        nc.vector.tensor_sub(dst, dst, ta)

    # ---- o-projection piece ----
    def oproj_tt(self, out_d, b, tt):
        nc = self.nc
        ob = self.xpool.tile([P, D], BF16, name="ob", tag="xc")
        for nq in range(D // 512):
            ps = self.ps_mm.tile([P, 512], F32, name="pso2", tag="mm")
            for h in range(NHL):
                nc.tensor.matmul(
                    ps,
                    lhsT=self.aoT[(b, h)][:, tt * P : (tt + 1) * P],
                    rhs=self.wo_sb[:, h, nq * 512 : (nq + 1) * 512],
                    start=(h == 0),
                    stop=(h == NHL - 1),
                )
            if nq % 2 == 0:
                nc.vector.tensor_copy(ob[:, nq * 512 : (nq + 1) * 512], ps)
            else:
                nc.scalar.copy(ob[:, nq * 512 : (nq + 1) * 512], ps)
            if nq % 2 == 1:
                nc.sync.dma_start(
                    out_d[
                        b * S + tt * P : b * S + (tt + 1) * P,
                        (nq - 1) * 512 : (nq + 1) * 512,
                    ],
                    ob[:, (nq - 1) * 512 : (nq + 1) * 512],
                )


def _proj_slot_groups(k, xt, b, h, with_v, skip_first_xc=False,
                      skip_first_wq1=False):
    """Yield emission closures for one head-slot's projections (chunks x
    4 weights, plus optionally the v projection groups)."""
    for ci in range(S // CHUNK):
        if not (skip_first_xc and ci == 0):
            yield lambda ci=ci: k.load_xc(xt, b, ci)
        for n in QKN:
            if skip_first_wq1 and ci == 0 and n == "wq1":
                continue
            yield lambda n=n, ci=ci: k.proj_qk_group(b, n, h, ci)
        if with_v:
            for s4 in range(CHUNK // P):
                yield lambda ci=ci, s4=s4: k.proj_v_group(b, ci, s4)


def _attn_head(k, b, h, bg_iter, post_pv=None):
    """Emit one head's attention (two q-halves), interleaving background
    closures (projections of the next head-slot / o-projection token tiles)
    between emission steps.  post_pv(qc_global) yields closures that depend on
    this head's PV output for 512-token chunk qc_global (the final
    o-projection); they are paced after subsequent steps."""
    nc = k.nc
    n_bg = getattr(bg_iter, "length", 0)
    emitted = 0
    pending = []
    steps_total = 2 * (2 * (KT + 1) + KT + 2 + 2)   # per-half emission steps
    steps_total *= 1
    step = 0
    total_steps = 2 * (2 * (KT + 1) + KT + 4)

    def bg_tick():
        nonlocal emitted, step
        step += 1
        if pending:
            pending.pop(0)()
        want = (step * n_bg) // total_steps
        while emitted < want:
            next(bg_iter.it)()
            emitted += 1

    for half in range(2):
        e1 = k.apool.tile([P, KT, QH], BF16, name="e1", tag="e1", bufs=1)
        e2 = k.apool.tile([P, KT, QH], F8, name="e2", tag="e2", bufs=1)
        l2 = k.apool.tile([P, QH], F16, name="l2", tag="l2", bufs=1)
        l1 = k.apool.tile([P, QH], F16, name="l1", tag="l1", bufs=1)
        # softmax-2 first (fp8 E2 -> held fp8 DoubleRow PV2), then softmax-1
        for si, ebuf, lt in ((1, e2, l2), (0, e1, l1)):
            tacc = k.apool.tile([P, QH], BF16, name="tacc", tag="tacc", bufs=2)
            for kt in range(KT):
                k.attn_qk_kt(b, si, h, half, kt, ebuf, tacc)
                bg_tick()
            if si == 1:
                pso2s = [k.attn_pv2(b, h, qc, e2) for qc in range(QH // QC)]
                bg_tick()
            k.attn_sums(si, h, tacc, lt)
            bg_tick()
        for qc in range(QH // QC):
            k.attn_pv1_merge(b, h, half, qc, e1, pso2s[qc])
            bg_tick()
            qc_global = half * (QH // QC) + qc
            if post_pv is not None:
                pending.extend(post_pv(qc_global))
            bg_tick()
    for fn in pending:
        fn()
    while emitted < n_bg:
        next(bg_iter.it)()
        emitted += 1


class _BG:
    def __init__(self, gens):
        items = [g for gen in gens for g in gen]
        self.it = iter(items)
        self.length = len(items)


def build_nc():
    nc = bass.Bass("TRN2", target_bir_lowering=False, debug=False)

    xt = nc.dram_tensor("xt", [D, T], BF16, kind="ExternalInput")
    wnames = ["wq1", "wk1", "wq2", "wk2", "wv"]
    w_d = {n: nc.dram_tensor(n, [P, KT * MD], BF16, kind="ExternalInput") for n in wnames}
    wo_d = nc.dram_tensor("wo", [P, NHL * D], BF16, kind="ExternalInput")
    loglam_d = nc.dram_tensor("loglam", [P, NHL + 1], F32, kind="ExternalInput")
    ones_d = nc.dram_tensor("ones", [P, P], BF16, kind="ExternalInput")
    out_d = nc.dram_tensor("out", [T, D], BF16, kind="ExternalOutput")

    with tile.TileContext(nc) as tc:
        with (
            tc.tile_pool(name="const", bufs=1) as cpool,
            tc.tile_pool(name="proj", bufs=1) as projpool,
            tc.tile_pool(name="xchunk", bufs=2) as xpool,
            tc.tile_pool(name="attn", bufs=2) as apool,
            tc.tile_pool(name="ps_score", bufs=2, space="PSUM") as ps_score,
            tc.tile_pool(name="ps_mm", bufs=2, space="PSUM") as ps_mm,
            tc.tile_pool(name="ps_sum", bufs=1, space="PSUM") as ps_sum,
        ):
            k = Kern(nc, tc, (cpool, projpool, xpool, apool,
                              ps_score, ps_mm, ps_sum))
            # DMA queues drain in emission order: put the first half-chunk of
            # x and the first-used weight at the head of the line so the PE
            # starts as soon as ~1.5MB have landed.
            k.load_xc(xt, 0, 0, half=0)
            k.load_w(w_d, "wq1", split=True)
            k.load_xc(xt, 0, 0, half=1)
            for n in ["wk1", "wq2", "wk2", "wv"]:
                k.load_w(w_d, n)
            k.load_consts(loglam_d, ones_d)

            # prologue: batch-0 head-0 projections + v(b0); the first chunk's
            # q1 projection runs as two half-width groups
            k.proj_qk_group(0, "wq1", 0, 0, half=0)
            k.proj_qk_group(0, "wq1", 0, 0, half=1)
            for fn in _proj_slot_groups(k, xt, 0, 0, with_v=True, skip_first_xc=True,
                                        skip_first_wq1=True):
                fn()
            k.load_wo(wo_d)
            # attn(b0,h0) x proj(b0,h1)
            _attn_head(k, 0, 0, _BG([_proj_slot_groups(k, xt, 0, 1, False)]))
            # attn(b0,h1) x proj(b1,h0)+v(b1)
            _attn_head(k, 0, 1, _BG([_proj_slot_groups(k, xt, 1, 0, True)]))
            # attn(b1,h0) x proj(b1,h1) + first half of oproj(b0)
            _attn_head(k, 1, 0, _BG([
                _proj_slot_groups(k, xt, 1, 1, False),
                [(lambda tt=tt: k.oproj_tt(out_d, 0, tt)) for tt in range(ST // 2)],
            ]))
            # attn(b1,h1) x oproj(b0); oproj(b1,tt) drains right after the PV
            # that completes its aoT columns, leaving almost no tail.
            def _drain_oproj_b1(qc):
                return [
                    (lambda tt=tt: k.oproj_tt(out_d, 1, tt))
                    for tt in range(4 * qc, 4 * qc + 4)
                ]

            _attn_head(
                k, 1, 1,
                _BG([[(lambda tt=tt: k.oproj_tt(out_d, 0, tt)) for tt in range(ST // 2, ST)]]),
                post_pv=_drain_oproj_b1,
            )

    _split_multi_waits(nc)
    return nc


_NC_CACHE = None


def _get_nc():
    global _NC_CACHE
    if _NC_CACHE is None:
        _NC_CACHE = build_nc()
    return _NC_CACHE


def _wlay(w_shard):
    """[MD, D] weight shard -> W.T laid out as the SBUF tile [128, KT*MD]."""
    bf = ml_dtypes.bfloat16
    wt = w_shard.T                                   # [D, MD]
    return np.ascontiguousarray(
        wt.reshape(KT, P, MD).transpose(1, 0, 2).reshape(P, KT * MD)
    ).astype(bf)


def _wolay(wo_shard):
    """[D, MD] o_w columns -> O.T laid out as the SBUF tile [128, NHL*D]."""
    bf = ml_dtypes.bfloat16
    wt = wo_shard.T                                  # [MD, D]
    return np.ascontiguousarray(
        wt.reshape(NHL, P, D).transpose(1, 0, 2).reshape(P, NHL * D)
    ).astype(bf)


def make_in_maps(inputs):
    bf = ml_dtypes.bfloat16
    x = np.asarray(inputs["x"], np.float32)
    lam = np.asarray(inputs["lambda_param"], np.float32)
    xt = np.ascontiguousarray(x.reshape(T, D).T).astype(bf)
    ones = np.ones((P, P), dtype=bf)

    in_maps = []
    for c in range(N_CORES):
        hs = slice(c * MD, (c + 1) * MD)
        m = {
            "xt": xt,
            "wq1": _wlay(np.asarray(inputs["q1_w"], np.float32)[hs, :]),
            "wk1": _wlay(np.asarray(inputs["k1_w"], np.float32)[hs, :]),
            "wq2": _wlay(np.asarray(inputs["q2_w"], np.float32)[hs, :]),
            "wk2": _wlay(np.asarray(inputs["k2_w"], np.float32)[hs, :]),
            "wv": _wlay(np.asarray(inputs["v_w"], np.float32)[hs, :]),
            "wo": _wolay(np.asarray(inputs["o_w"], np.float32)[:, hs]),
            "loglam": np.tile(np.concatenate([np.log(lam[c * NHL : (c + 1) * NHL]) - LNC, [-LNC]])[None, :], (P, 1)).astype(np.float32),
            "ones": ones,
        }
        in_maps.append(m)
    return in_maps


def kernel(**inputs):
    in_maps = make_in_maps(inputs)
    nc = _get_nc()
    res = bass_utils.run_bass_kernel_spmd(nc, in_maps, core_ids=list(range(N_CORES)))
    acc = np.zeros((T, D), np.float64)
    for r in res.results:
        acc += np.asarray(r["out"], np.float64)
    return acc.reshape(B, S, D).astype(np.float32)


if __name__ == "__main__":
    nc = build_nc()
    print("built OK")


# revision 19
# speedup vs baseline: 1.4752x; 1.0009x over previous
"""Differential attention (DiffAttn) Trainium2 kernel, 8-core tensor-parallel.

Reference computation (per batch b, head h):
    q1,k1,q2,k2,v = x @ W*.T          (x: [B,S,D], W: [D,D], 16 heads x 128)
    a1 = softmax(q1 k1^T / sqrt(dh)); a2 = softmax(q2 k2^T / sqrt(dh))
    out = ((a1 - lam_h * a2) @ v) @ o_w.T

Sharding: tensor-parallel over heads. Core c owns heads {2c, 2c+1} (d_model
slice 256c:256c+256 of the projection outputs).  Each core computes a partial
o-projection output over its 256 input dims; the host sums the 8 partials.

Device-side layout choices:
  - x is passed pre-transposed (xt = x.T, [D, B*S]) so projections can run
    as  out.T[m, tok] = W_shard @ x.T  with the weight shard (host
    pre-transposed) as the stationary operand -> q/k tiles land in
    [head_dim(part), token(free)] layout, which feeds QK^T directly.
  - v is produced in natural [token, dim] layout (lhsT = x.T chunks) so it can
    be the stationary operand of the PV matmul.
  - scores are computed TRANSPOSED from the start: the k-block is the
    stationary operand and q streams, so the score tile lands as
    [key(part), query(free)].  exp'd tiles (E) then feed the PV matmul
    directly as the moving operand -- the PE-transpose pass of the previous
    design is gone entirely.
  - softmax sums in this layout run across partitions+tiles: a serial DVE
    accumulator T += E_kt trails the exp stream, then one ones-matmul
    reduces T across partitions into PSUM (every output row = the sum,
    broadcast for free).  r1 = 1/s1 and g = -lam*s1*r2 are [128, q] fp16 row
    vectors; E_comb = E1 + g*E2 in-place; the 1/s1 normalization rides the
    PV PSUM->SBUF copy as a tensor-tensor multiply.
  - attention is processed in q-halves of 1024 so the 32 E tiles fit SBUF.
  - all matmul inputs are bf16; PSUM accumulation is fp32; the o-proj output
    is stored bf16 (host accumulates the 8 partials in fp64).

Engine balance: attention is elementwise-bound (ACT exp, DVE accum/combine),
the projections are PE-bound.  The emission order software-pipelines them:

    proj(b0,h0)+v(b0) | attn(b0,h0) x proj(b0,h1) | attn(b0,h1) x proj(b1,h0)+v(b1)
    | attn(b1,h0) x proj(b1,h1) + oproj(b0)/2 | attn(b1,h1) x oproj(b0)/2; oproj(b1) drains
"""

import math

import numpy as np
import ml_dtypes

import concourse.bass as bass
import concourse.mybir as mybir
import concourse.tile as tile
from concourse import bass_utils

BF16 = mybir.dt.bfloat16
F16 = mybir.dt.float16
F8 = mybir.dt.float8e4
F32 = mybir.dt.float32

P = 128           # partitions / head_dim / PE tile
D = 2048          # d_model
B = 2
S = 2048          # seq len
T = B * S         # 4096 tokens
NH = 16           # total heads
NHL = 2           # heads per core
MD = NHL * P      # per-core projection dim (256)
KT = D // P       # 16 contraction tiles over d_model
ST = S // P       # 16 token tiles per batch
N_CORES = 8
CHUNK = 512       # token chunk for projection x streaming
QH = 1024         # q-half width for attention
QC = 512          # PV q-chunk
SCALE = 1.0 / math.sqrt(P)
LNSC = 2.0 ** -11          # pre-scale inside ln() so fp16 logs sit near 0
LNC = 11.0 * math.log(2.0)  # ln(1/LNSC)
QKN = ["wq1", "wk1", "wq2", "wk2"]
TACC_SPLIT = 8             # k-tiles 0..7 accumulate on DVE, 8..15 on GPSIMD
COMB_SPLIT = 4             # k-tiles 0..3 combine on GPSIMD, 4..15 on DVE

_mult = mybir.AluOpType.mult
_add = mybir.AluOpType.add


def _split_multi_waits(nc):
    """This walrus build accepts at most ONE sync-wait per instruction
    (codegen: "Too many sync wait commands").  Tile attaches one wait per
    upstream proc, so split the extras onto same-engine NOP carriers placed
    immediately before the instruction — the engine stalls on each carrier in
    turn, which is sequentially equivalent."""
    n = 0
    for bb in nc.main_func.blocks:
        out = []
        for ins in bb.instructions:
            si = getattr(ins, "sync_info", None)
            waits = list(si.on_wait) if si is not None and si.on_wait else []
            if len(waits) > 1:
                for w in waits[:-1]:
                    n += 1
                    out.append(
                        mybir.InstNoOp(
                            name=f"{ins.name}-wsplit{n}",
                            engine=ins.engine,
                            sync_info=mybir.SyncInfo(on_wait=[w], on_update=[]),
                            bass_nofuse=True,
                        )
                    )
                si.on_wait = waits[-1:]
            out.append(ins)
        bb.instructions[:] = out


class Kern:
    """Holds pools/constants; methods emit one group of instructions each.
    The driver (build) calls them in a software-pipelined order."""

    def __init__(self, nc, tc, pools):
        self.nc = nc
        self.tc = tc
        (self.cpool, self.projpool, self.xpool, self.apool,
         self.ps_score, self.ps_mm, self.ps_sum) = pools
        self.qk = {}      # (b, n, h) -> tile (slots shared across b via tags)
        self.vbuf = {}    # b -> tile
        self.vbuf8 = {}   # b -> fp8 copy for the DoubleRow PV2
        self.aoT = {}     # (b, h) -> tile
        self.xc_cur = None
        self.w_sb = {}

    def load_w(self, w_d, n, split=False):
        t = self.cpool.tile([P, KT, MD], BF16, name=f"{n}_sb")
        src_ap = w_d[n].rearrange("p (kt m) -> p kt m", m=MD)
        if split:
            self.nc.sync.dma_start(t[:, : KT // 2], src_ap[:, : KT // 2])
            self.nc.sync.dma_start(t[:, KT // 2 :], src_ap[:, KT // 2 :])
        else:
            self.nc.sync.dma_start(t, src_ap)
        self.w_sb[n] = t

    def load_consts(self, loglam_d, ones_d):
        nc = self.nc
        self.loglam_sb = self.cpool.tile([P, NHL + 1], F32, name="loglam_sb")
        nc.sync.dma_start(self.loglam_sb, loglam_d.ap())
        self.ones_sb = self.cpool.tile([P, P], BF16, name="ones_sb")
        nc.sync.dma_start(self.ones_sb, ones_d.ap())

    def load_wo(self, wo_d):
        self.wo_sb = self.cpool.tile([P, NHL, D], BF16, name="wo_sb")
        self.nc.sync.dma_start(self.wo_sb, wo_d.rearrange("p (h n) -> p h n", n=D))

    # ---- projection pieces ----
    def load_xc(self, xt, b, ci, half=None):
        tok0 = b * S + ci * CHUNK
        if half is None:
            xc = self.xpool.tile([P, KT, CHUNK], BF16, name="xc", tag="xc")
            self.nc.sync.dma_start(
                xc, xt[:, tok0 : tok0 + CHUNK].rearrange("(kt p) n -> p kt n", p=P)
            )
            self.xc_cur = xc
        else:
            # first-chunk fast path: land each half as its own DMA so the
            # first matmuls only wait on 0.5MB of x
            if half == 0:
                self.xc_cur = self.xpool.tile([P, KT, CHUNK], BF16, name="xc", tag="xc")
            hw = CHUNK // 2
            self.nc.sync.dma_start(
                self.xc_cur[:, :, half * hw : (half + 1) * hw],
                xt[:, tok0 + half * hw : tok0 + (half + 1) * hw].rearrange(
                    "(kt p) n -> p kt n", p=P
                ),
            )

    def proj_qk_group(self, b, n, h, ci, half=None):
        """One (weight, head) projection for one CHUNK-token chunk: 16 matmuls
        accumulated in PSUM, then a DVE cast into the persistent qk tile."""
        nc = self.nc
        key = (b, n, h)
        if key not in self.qk:
            self.qk[key] = self.projpool.tile(
                [P, S], BF16, name=f"{n}h{h}b{b}", tag=f"{n}h{h}"
            )
        ps = self.ps_mm.tile([P, 512], F32, name="psp", tag="mm")
        xc = self.xc_cur
        lo, sz = (0, CHUNK) if half is None else (half * (CHUNK // 2), CHUNK // 2)
        for kt in range(KT):
            nc.tensor.matmul(
                ps[:, :sz],
                lhsT=self.w_sb[n][:, kt, h * P : (h + 1) * P],
                rhs=xc[:, kt, lo : lo + sz],
                start=(kt == 0),
                stop=(kt == KT - 1),
            )
        nc.vector.tensor_copy(
            self.qk[key][:, ci * CHUNK + lo : ci * CHUNK + lo + sz], ps[:, :sz]
        )

    def proj_v_group(self, b, ci, s4):
        nc = self.nc
        if b not in self.vbuf:
            self.vbuf[b] = self.projpool.tile(
                [P, ST, MD], BF16, name=f"vbuf{b}", tag=f"vbuf{b}"
            )
            self.vbuf8[b] = self.projpool.tile(
                [P, ST, MD], F8, name=f"vbuf8{b}", tag=f"vbuf8{b}"
            )
        ps = self.ps_mm.tile([P, 512], F32, name="psv", tag="mm")
        xc = self.xc_cur
        for kt in range(KT):
            nc.tensor.matmul(
                ps[:, :MD],
                lhsT=xc[:, kt, s4 * P : (s4 + 1) * P],
                rhs=self.w_sb["wv"][:, kt, :],
                start=(kt == 0),
                stop=(kt == KT - 1),
            )
        blk = ci * (CHUNK // P) + s4
        nc.vector.tensor_copy(self.vbuf[b][:, blk, :], ps[:, :MD])
        nc.scalar.copy(self.vbuf8[b][:, blk, :], ps[:, :MD])

    # ---- attention pieces (scores in [k, q] layout) ----
    def attn_qk_kt(self, b, si, h, half, kt, ebuf, tacc):
        """Scores+exp for one (softmax, k-tile): psum[k=128, q=1024] via two
        512-wide matmuls with the k-block stationary, exp into ebuf[:, kt, :]
        (bf16 for softmax-1, fp8 for softmax-2), and the serial DVE
        sum-accumulator T += E_kt trailing the exp stream."""
        nc = self.nc
        qn, kn = ("wq1", "wk1") if si == 0 else ("wq2", "wk2")
        qh_t = self.qk[(b, qn, h)]
        kh_t = self.qk[(b, kn, h)]
        qlo = half * QH
        ps = self.ps_score.tile([P, QH], F32, name="pss", tag="score")
        for j in range(QH // 512):
            nc.tensor.matmul(
                ps[:, j * 512 : (j + 1) * 512],
                lhsT=kh_t[:, kt * P : (kt + 1) * P],
                rhs=qh_t[:, qlo + j * 512 : qlo + (j + 1) * 512],
                start=True,
                stop=True,
            )
        nc.scalar.activation(
            ebuf[:, kt, :], ps, mybir.ActivationFunctionType.Exp, scale=SCALE
        )
        if kt == 1:
            nc.vector.tensor_add(tacc, ebuf[:, 0, :], ebuf[:, 1, :])
        elif kt > 1:
            nc.vector.tensor_add(tacc, tacc, ebuf[:, kt, :])

    def attn_sums(self, si, h, tacc):
        """Partition-reduce T via a ones-matmul into PSUM (broadcast sum),
        then L = ln(s * 2^-11) fp16 on ACT, and the fp16 column scales:
        softmax-2: gl2 = exp(-L2 + ln(lam) - LNC) = lam/s2
        softmax-1: r1  = exp(-L1 - LNC) = 1/s1"""
        nc = self.nc
        pssum = self.ps_score.tile([P, QH], F32, name="pssum", tag="score")
        for j in range(QH // 512):
            sl = slice(j * 512, (j + 1) * 512)
            nc.tensor.matmul(pssum[:, sl], lhsT=self.ones_sb, rhs=tacc[:, sl],
                             start=True, stop=True)
        ltile = self.apool.tile([P, QH], F16, name="lt", tag="scr", bufs=1)
        nc.scalar.activation(ltile, pssum, mybir.ActivationFunctionType.Ln,
                             scale=LNSC)
        with nc.allow_low_precision(reason="fp16 softmax normalizers"):
            if si == 1:
                gl2 = self.apool.tile([P, QH], F16, name="gl2", tag="gl2", bufs=1)
                nc.scalar.activation(gl2, ltile,
                                     mybir.ActivationFunctionType.Exp,
                                     scale=-1.0,
                                     bias=self.loglam_sb[:, h : h + 1])
                self.gl2_cur = gl2
            else:
                r1 = self.apool.tile([P, QH], F16, name="r1", tag="r1", bufs=1)
                nc.scalar.activation(r1, ltile,
                                     mybir.ActivationFunctionType.Exp,
                                     scale=-1.0,
                                     bias=self.loglam_sb[:, NHL : NHL + 1])
                self.r1_cur = r1

    def attn_pv2(self, b, h, qc, e2buf):
        """lam-weighted softmax-2 PV in fp8 DoubleRow: contracts k-tile PAIRS
        (256 per matmul) at 2x rate.  Result is held in PSUM (tag pv2) until
        the merge after PV1."""
        nc = self.nc
        pso2 = self.ps_mm.tile([P, 512], F32, name="pso2", tag="pv2", bufs=2)
        for t in range(ST // 2):
            nc.tensor.matmul(
                pso2[:, :QC],
                lhsT=self.vbuf8[b][:, 2 * t : 2 * t + 2, h * P : (h + 1) * P],
                rhs=e2buf[:, 2 * t : 2 * t + 2, qc * QC : (qc + 1) * QC],
                start=(t == 0),
                stop=(t == ST // 2 - 1),
                perf_mode=mybir.MatmulPerfMode.DoubleRow,
            )
        return pso2

    def attn_pv1_merge(self, b, h, half, qc, e1buf, pso2):
        """Softmax-1 PV (bf16) + merge: aoT = pso1*r1 - pso2*gl2."""
        nc = self.nc
        if (b, h) not in self.aoT:
            self.aoT[(b, h)] = self.projpool.tile(
                [P, S], BF16, name=f"aoT{b}{h}", tag=f"aoT{b}{h}"
            )
        pso1 = self.ps_mm.tile([P, 512], F32, name="pso1", tag="mm")
        for kt in range(ST):
            nc.tensor.matmul(
                pso1[:, :QC],
                lhsT=self.vbuf[b][:, kt, h * P : (h + 1) * P],
                rhs=e1buf[:, kt, qc * QC : (qc + 1) * QC],
                start=(kt == 0),
                stop=(kt == ST - 1),
            )
        dst = self.aoT[(b, h)][:, half * QH + qc * QC : half * QH + (qc + 1) * QC]
        ta = self.apool.tile([P, QC], BF16, name="ta", tag="scr", bufs=1)
        nc.vector.tensor_mul(ta, pso2[:, :QC], self.gl2_cur[:, qc * QC : (qc + 1) * QC])
        nc.vector.tensor_mul(dst, pso1[:, :QC], self.r1_cur[:, qc * QC : (qc + 1) * QC])
        nc.vector.tensor_sub(dst, dst, ta)

    # ---- o-projection piece ----
    def oproj_tt(self, out_d, b, tt):
        nc = self.nc
        ob = self.xpool.tile([P, D], BF16, name="ob", tag="xc")
        for nq in range(D // 512):
            ps = self.ps_mm.tile([P, 512], F32, name="pso2", tag="mm")
            for h in range(NHL):
                nc.tensor.matmul(
                    ps,
                    lhsT=self.aoT[(b, h)][:, tt * P : (tt + 1) * P],
                    rhs=self.wo_sb[:, h, nq * 512 : (nq + 1) * 512],
                    start=(h == 0),
                    stop=(h == NHL - 1),
                )
            if nq % 2 == 0:
                nc.vector.tensor_copy(ob[:, nq * 512 : (nq + 1) * 512], ps)
            else:
                nc.scalar.copy(ob[:, nq * 512 : (nq + 1) * 512], ps)
            if nq % 2 == 1:
                nc.sync.dma_start(
                    out_d[
                        b * S + tt * P : b * S + (tt + 1) * P,
                        (nq - 1) * 512 : (nq + 1) * 512,
                    ],
                    ob[:, (nq - 1) * 512 : (nq + 1) * 512],
                )


def _proj_slot_groups(k, xt, b, h, with_v, skip_first_xc=False,
                      skip_first_wq1=False):
    """Yield emission closures for one head-slot's projections (chunks x
    4 weights, plus optionally the v projection groups)."""
    for ci in range(S // CHUNK):
        if not (skip_first_xc and ci == 0):
            yield lambda ci=ci: k.load_xc(xt, b, ci)
        for n in QKN:
            if skip_first_wq1 and ci == 0 and n == "wq1":
                continue
            yield lambda n=n, ci=ci: k.proj_qk_group(b, n, h, ci)
        if with_v:
            for s4 in range(CHUNK // P):
                yield lambda ci=ci, s4=s4: k.proj_v_group(b, ci, s4)


def _attn_head(k, b, h, bg_iter, post_pv=None):
    """Emit one head's attention (two q-halves), interleaving background
    closures (projections of the next head-slot / o-projection token tiles)
    between emission steps.  post_pv(qc_global) yields closures that depend on
    this head's PV output for 512-token chunk qc_global (the final
    o-projection); they are paced after subsequent steps."""
    nc = k.nc
    n_bg = getattr(bg_iter, "length", 0)
    emitted = 0
    pending = []
    steps_total = 2 * (2 * (KT + 1) + KT + 2 + 2)   # per-half emission steps
    steps_total *= 1
    step = 0
    total_steps = 2 * (2 * (KT + 1) + KT + 4)

    def bg_tick():
        nonlocal emitted, step
        step += 1
        if pending:
            pending.pop(0)()
        want = (step * n_bg) // total_steps
        while emitted < want:
            next(bg_iter.it)()
            emitted += 1

    for half in range(2):
        e1 = k.apool.tile([P, KT, QH], BF16, name="e1", tag="e1", bufs=1)
        e2 = k.apool.tile([P, KT, QH], F8, name="e2", tag="e2", bufs=1)
        l2 = None
        l1 = None
        # softmax-2 first (fp8 E2 -> held fp8 DoubleRow PV2), then softmax-1
        for si, ebuf in ((1, e2), (0, e1)):
            tacc = k.apool.tile([P, QH], BF16, name="tacc", tag="scr", bufs=1)
            for kt in range(KT):
                k.attn_qk_kt(b, si, h, half, kt, ebuf, tacc)
                bg_tick()
            if si == 1:
                pso2s = [k.attn_pv2(b, h, qc, e2) for qc in range(QH // QC)]
                bg_tick()
            k.attn_sums(si, h, tacc)
            bg_tick()
        for qc in range(QH // QC):
            k.attn_pv1_merge(b, h, half, qc, e1, pso2s[qc])
            bg_tick()
            qc_global = half * (QH // QC) + qc
            if post_pv is not None:
                pending.extend(post_pv(qc_global))
            bg_tick()
    for fn in pending:
        fn()
    while emitted < n_bg:
        next(bg_iter.it)()
        emitted += 1


class _BG:
    def __init__(self, gens):
        items = [g for gen in gens for g in gen]
        self.it = iter(items)
        self.length = len(items)


def build_nc():
    nc = bass.Bass("TRN2", target_bir_lowering=False, debug=False)

    xt = nc.dram_tensor("xt", [D, T], BF16, kind="ExternalInput")
    wnames = ["wq1", "wk1", "wq2", "wk2", "wv"]
    w_d = {n: nc.dram_tensor(n, [P, KT * MD], BF16, kind="ExternalInput") for n in wnames}
    wo_d = nc.dram_tensor("wo", [P, NHL * D], BF16, kind="ExternalInput")
    loglam_d = nc.dram_tensor("loglam", [P, NHL + 1], F32, kind="ExternalInput")
    ones_d = nc.dram_tensor("ones", [P, P], BF16, kind="ExternalInput")
    out_d = nc.dram_tensor("out", [T, D], BF16, kind="ExternalOutput")

    with tile.TileContext(nc) as tc:
        with (
            tc.tile_pool(name="const", bufs=1) as cpool,
            tc.tile_pool(name="proj", bufs=1) as projpool,
            tc.tile_pool(name="xchunk", bufs=2) as xpool,
            tc.tile_pool(name="attn", bufs=2) as apool,
            tc.tile_pool(name="ps_score", bufs=2, space="PSUM") as ps_score,
            tc.tile_pool(name="ps_mm", bufs=2, space="PSUM") as ps_mm,
            tc.tile_pool(name="ps_sum", bufs=1, space="PSUM") as ps_sum,
        ):
            k = Kern(nc, tc, (cpool, projpool, xpool, apool,
                              ps_score, ps_mm, ps_sum))
            # DMA queues drain in emission order: put the first half-chunk of
            # x and the first-used weight at the head of the line so the PE
            # starts as soon as ~1.5MB have landed.
            k.load_xc(xt, 0, 0, half=0)
            k.load_w(w_d, "wq1", split=True)
            k.load_xc(xt, 0, 0, half=1)
            for n in ["wk1", "wq2", "wk2", "wv"]:
                k.load_w(w_d, n)
            k.load_consts(loglam_d, ones_d)

            # prologue: batch-0 head-0 projections + v(b0); the first chunk's
            # q1 projection runs as two half-width groups
            k.proj_qk_group(0, "wq1", 0, 0, half=0)
            k.proj_qk_group(0, "wq1", 0, 0, half=1)
            for fn in _proj_slot_groups(k, xt, 0, 0, with_v=True, skip_first_xc=True,
                                        skip_first_wq1=True):
                fn()
            k.load_wo(wo_d)
            # attn(b0,h0) x proj(b0,h1)
            _attn_head(k, 0, 0, _BG([_proj_slot_groups(k, xt, 0, 1, False)]))
            # attn(b0,h1) x proj(b1,h0)+v(b1)
            _attn_head(k, 0, 1, _BG([_proj_slot_groups(k, xt, 1, 0, True)]))
            # attn(b1,h0) x proj(b1,h1) + first half of oproj(b0)
            _attn_head(k, 1, 0, _BG([
                _proj_slot_groups(k, xt, 1, 1, False),
                [(lambda tt=tt: k.oproj_tt(out_d, 0, tt)) for tt in range(ST // 2)],
            ]))
            # attn(b1,h1) x oproj(b0); oproj(b1,tt) drains right after the PV
            # that completes its aoT columns, leaving almost no tail.
            def _drain_oproj_b1(qc):
                return [
                    (lambda tt=tt: k.oproj_tt(out_d, 1, tt))
                    for tt in range(4 * qc, 4 * qc + 4)
                ]

            _attn_head(
                k, 1, 1,
                _BG([[(lambda tt=tt: k.oproj_tt(out_d, 0, tt)) for tt in range(ST // 2, ST)]]),
                post_pv=_drain_oproj_b1,
            )

    _split_multi_waits(nc)
    return nc


_NC_CACHE = None


def _get_nc():
    global _NC_CACHE
    if _NC_CACHE is None:
        _NC_CACHE = build_nc()
    return _NC_CACHE


def _wlay(w_shard):
    """[MD, D] weight shard -> W.T laid out as the SBUF tile [128, KT*MD]."""
    bf = ml_dtypes.bfloat16
    wt = w_shard.T                                   # [D, MD]
    return np.ascontiguousarray(
        wt.reshape(KT, P, MD).transpose(1, 0, 2).reshape(P, KT * MD)
    ).astype(bf)


def _wolay(wo_shard):
    """[D, MD] o_w columns -> O.T laid out as the SBUF tile [128, NHL*D]."""
    bf = ml_dtypes.bfloat16
    wt = wo_shard.T                                  # [MD, D]
    return np.ascontiguousarray(
        wt.reshape(NHL, P, D).transpose(1, 0, 2).reshape(P, NHL * D)
    ).astype(bf)


def make_in_maps(inputs):
    bf = ml_dtypes.bfloat16
    x = np.asarray(inputs["x"], np.float32)
    lam = np.asarray(inputs["lambda_param"], np.float32)
    xt = np.ascontiguousarray(x.reshape(T, D).T).astype(bf)
    ones = np.ones((P, P), dtype=bf)

    in_maps = []
    for c in range(N_CORES):
        hs = slice(c * MD, (c + 1) * MD)
        m = {
            "xt": xt,
            "wq1": _wlay(np.asarray(inputs["q1_w"], np.float32)[hs, :]),
            "wk1": _wlay(np.asarray(inputs["k1_w"], np.float32)[hs, :]),
            "wq2": _wlay(np.asarray(inputs["q2_w"], np.float32)[hs, :]),
            "wk2": _wlay(np.asarray(inputs["k2_w"], np.float32)[hs, :]),
            "wv": _wlay(np.asarray(inputs["v_w"], np.float32)[hs, :]),
            "wo": _wolay(np.asarray(inputs["o_w"], np.float32)[:, hs]),
            "loglam": np.tile(np.concatenate([np.log(lam[c * NHL : (c + 1) * NHL]) - LNC, [-LNC]])[None, :], (P, 1)).astype(np.float32),
            "ones": ones,
        }
        in_maps.append(m)
    return in_maps


def kernel(**inputs):
    in_maps = make_in_maps(inputs)
    nc = _get_nc()
    res = bass_utils.run_bass_kernel_spmd(nc, in_maps, core_ids=list(range(N_CORES)))
    acc = np.zeros((T, D), np.float64)
    for r in res.results:
        acc += np.asarray(r["out"], np.float64)
    return acc.reshape(B, S, D).astype(np.float32)


if __name__ == "__main__":
    nc = build_nc()
    print("built OK")
